# revision 3
# baseline (speedup 1.0000x reference)
"""Distributed kNN-softmax kernel for Trainium2 (8 NeuronCores).

Computes softmax_n(dist[b, n]) where
    dist[b, n] = sum_{l,a} (raw_phrases[n, l, a] - x[b, l, a])**2
for x [16, 32, 128] and raw_phrases [50000, 32, 128].

Strategy (memory-bound regime):
  - Shard raw_phrases along N across the 8 cores (6250 rows each); x is
    replicated. Since softmax over n is invariant to per-b shifts, the
    x2[b] term is dropped entirely: dist'[b,n] = r2[n] - 2*xr[b,n].
  - Host prep only re-lays-out inputs: each shard is transposed to
    [4096, 6250] so the contraction dim (la) lands on SBUF partitions,
    and x is transposed/scaled to -2*x^T [4096, 16].
  - Per core: stream r^T in [128 x 2048] slabs; for each k-chunk the PE
    accumulates (-2x)^T . r into PSUM [16, n] and a ones-matmul over the
    squared slab adds r2[n] broadcast to all 16 rows — so PSUM holds
    dist' directly. fp32r (fp22 mantissa truncation) runs the PE at full
    rate for N>=256.
  - Two-pass softmax: per-core row max (pass A), exp+sum (pass B, fused
    accumulate on the scalar engine), then one 8-core AllGather of the
    32 per-core stats floats, on-device global max/denominator combine,
    and a per-row scale of the local shard.
"""

import numpy as np
from contextlib import ExitStack

import concourse.tile as tile
from concourse import bacc, mybir
from concourse.bass_utils import run_bass_kernel_spmd

N_CORES = 8
B = 16
N_REF = 50000
D = 4096  # 32 * 128
SHARD = N_REF // N_CORES  # 6250
K_TILES = D // 128  # 32
N_SLAB = 2048
N_TILE = 512  # max fp32 moving free dim per matmul
F32 = mybir.dt.float32
F32R = mybir.dt.float32r

_CACHE = {}


def _slabs():
    out = []
    n0 = 0
    while n0 < SHARD:
        out.append((n0, min(N_SLAB, SHARD - n0)))
        n0 += N_SLAB
    return out


def build():
    nc = bacc.Bacc(
        "TRN2",
        target_bir_lowering=False,
        debug=False,
        num_devices=N_CORES,
        enable_asserts=False,
    )
    rt = nc.dram_tensor("rt", [D, SHARD], F32R, kind="ExternalInput").ap()
    xm2t = nc.dram_tensor("xm2t", [D, B], F32R, kind="ExternalInput").ap()
    onesd = nc.dram_tensor("onesd", [128, B], F32R, kind="ExternalInput").ap()
    out = nc.dram_tensor("out", [B, SHARD], F32, kind="ExternalOutput").ap()
    cc_in = nc.dram_tensor("cc_in", [2 * B], F32, kind="Internal")
    cc_out = nc.dram_tensor(
        "cc_out", [N_CORES * 2 * B], F32, kind="Internal", addr_space="Shared"
    )

    AX = mybir.AxisListType.X
    EXP = mybir.ActivationFunctionType.Exp

    with tile.TileContext(nc) as tc, ExitStack() as ctx:
        consts = ctx.enter_context(tc.tile_pool(name="consts", bufs=1))
        rpool = ctx.enter_context(tc.tile_pool(name="rpool", bufs=3))
        sqpool = ctx.enter_context(tc.tile_pool(name="sqpool", bufs=3))
        pspool = ctx.enter_context(tc.tile_pool(name="pspool", bufs=8, space="PSUM"))
        spool = ctx.enter_context(tc.tile_pool(name="spool", bufs=1))
        stat = ctx.enter_context(tc.tile_pool(name="stat", bufs=1))

        # stationary operands
        xw = consts.tile([128, K_TILES, B], F32R)  # xw[p, k, b] = -2*x[b, k*128+p]
        nc.sync.dma_start(out=xw[:], in_=xm2t.rearrange("(k p) b -> p k b", p=128))
        ones = consts.tile([128, B], F32R)
        nc.sync.dma_start(out=ones[:], in_=onesd)

        dist = spool.tile([B, SHARD], F32)  # dist', later exp values
        n_tiles_total = sum((w + N_TILE - 1) // N_TILE for _, w in _slabs())
        mparts = stat.tile([B, n_tiles_total], F32)

        tcount = 0
        for n0, w in _slabs():
            subtiles = []
            j0 = 0
            while j0 < w:
                subtiles.append((j0, min(N_TILE, w - j0)))
                j0 += N_TILE
            ps = [
                pspool.tile([B, N_TILE], F32, tag="ps", name=f"ps{j}")
                for j in range(len(subtiles))
            ]
            for k in range(K_TILES):
                rt_t = rpool.tile([128, N_SLAB], F32R, tag="rt")
                nc.sync.dma_start(
                    out=rt_t[:, :w], in_=rt[k * 128 : (k + 1) * 128, n0 : n0 + w]
                )
                sq_t = sqpool.tile([128, N_SLAB], F32R, tag="sq")
                if k % 2 == 0:
                    nc.scalar.square(sq_t[:, :w], rt_t[:, :w])
                else:
                    nc.vector.tensor_mul(sq_t[:, :w], rt_t[:, :w], rt_t[:, :w])
                for j, (j0, jw) in enumerate(subtiles):
                    nc.tensor.matmul(
                        ps[j][:, :jw],
                        lhsT=xw[:, k, :],
                        rhs=rt_t[:, j0 : j0 + jw],
                        start=(k == 0),
                        stop=False,
                        skip_group_check=True,
                    )
                    nc.tensor.matmul(
                        ps[j][:, :jw],
                        lhsT=ones[:],
                        rhs=sq_t[:, j0 : j0 + jw],
                        start=False,
                        stop=(k == K_TILES - 1),
                        skip_group_check=True,
                    )
            for j, (j0, jw) in enumerate(subtiles):
                nc.scalar.copy(dist[:, n0 + j0 : n0 + j0 + jw], ps[j][:, :jw])
                nc.vector.reduce_max(
                    mparts[:, tcount : tcount + 1], ps[j][:, :jw], axis=AX
                )
                tcount += 1

        # local stats
        mloc = stat.tile([B, 1], F32)
        nc.vector.reduce_max(mloc[:], mparts[:, :tcount], axis=AX)
        negm = stat.tile([B, 1], F32)
        nc.vector.tensor_scalar_mul(negm[:], mloc[:], -1.0)
        sloc = stat.tile([B, 1], F32)
        # e = exp(dist - mloc), sloc = sum_n e   (fused accumulate)
        nc.scalar.activation(
            out=dist[:], in_=dist[:], func=EXP, bias=negm[:], scale=1.0,
            accum_out=sloc[:],
        )

        # distribute stats: [mloc, sloc] -> AllGather over the 8 cores
        stats2 = stat.tile([B, 2], F32)
        nc.vector.tensor_copy(stats2[:, 0:1], mloc[:])
        nc.vector.tensor_copy(stats2[:, 1:2], sloc[:])
        nc.sync.dma_start(
            out=cc_in.ap().rearrange("(b j) -> b j", j=2), in_=stats2[:]
        )
        nc.gpsimd.collective_compute(
            "AllGather",
            mybir.AluOpType.bypass,
            replica_groups=[list(range(N_CORES))],
            ins=[cc_in.ap()],
            outs=[cc_out.ap()],
        )
        gath = cc_out.ap().rearrange("(c b j) -> b j c", c=N_CORES, j=2)
        mall = stat.tile([B, N_CORES], F32)
        nc.sync.dma_start(out=mall[:], in_=gath[:, 0, :])
        sall = stat.tile([B, N_CORES], F32)
        nc.sync.dma_start(out=sall[:], in_=gath[:, 1, :])

        # global max and denominator: Z = sum_c s_c * exp(m_c - M)
        gmax = stat.tile([B, 1], F32)
        nc.vector.reduce_max(gmax[:], mall[:], axis=AX)
        dall = stat.tile([B, N_CORES], F32)
        nc.vector.tensor_scalar_sub(dall[:], mall[:], gmax[:])
        wexp = stat.tile([B, N_CORES], F32)
        nc.scalar.activation(out=wexp[:], in_=dall[:], func=EXP)
        wz = stat.tile([B, N_CORES], F32)
        nc.vector.tensor_mul(wz[:], wexp[:], sall[:])
        zsum = stat.tile([B, 1], F32)
        nc.vector.reduce_sum(zsum[:], wz[:], axis=AX)
        zinv = stat.tile([B, 1], F32)
        nc.vector.reciprocal(zinv[:], zsum[:])
        # alpha = exp(mloc - M) / Z
        dm = stat.tile([B, 1], F32)
        nc.vector.tensor_sub(dm[:], mloc[:], gmax[:])
        edm = stat.tile([B, 1], F32)
        nc.scalar.activation(out=edm[:], in_=dm[:], func=EXP)
        alpha = stat.tile([B, 1], F32)
        nc.vector.tensor_mul(alpha[:], edm[:], zinv[:])

        nc.vector.tensor_scalar_mul(dist[:], dist[:], alpha[:])
        nc.sync.dma_start(out=out, in_=dist[:])

    nc.compile()
    return nc


def _prep_inputs(x, raw_phrases):
    x2d = np.asarray(x, dtype=np.float32).reshape(B, D)
    xm2t = np.ascontiguousarray((-2.0 * x2d).T)  # [D, B]
    r2d = np.asarray(raw_phrases, dtype=np.float32).reshape(N_REF, D)
    in_maps = []
    for c in range(N_CORES):
        rt_c = np.ascontiguousarray(r2d[c * SHARD : (c + 1) * SHARD].T)  # [D, SHARD]
        in_maps.append({"rt": rt_c, "xm2t": xm2t,
                        "onesd": np.ones((128, B), np.float32)})
    return in_maps


def run(x, raw_phrases, **run_kwargs):
    """Build (cached), shard, execute on 8 cores, gather. Returns
    (full_output [16, 50000] f32, BassKernelResults)."""
    if "nc" not in _CACHE:
        _CACHE["nc"] = build()
    nc = _CACHE["nc"]
    in_maps = _prep_inputs(x, raw_phrases)
    res = run_bass_kernel_spmd(nc, in_maps, core_ids=list(range(N_CORES)), **run_kwargs)
    full = np.concatenate([res.results[c]["out"] for c in range(N_CORES)], axis=1)
    return full, res


def kernel(x, raw_phrases):
    full, _ = run(x, raw_phrases)
    return full
